# revision 1
# baseline (speedup 1.0000x reference)
"""Trainium2 Bass kernel for a bidirectional RNN language model.

Model: emb = embedding[input_batch]; two 16-wide tanh RNN scans (L->R and
R->L) over 128 steps; logits = [hLR, hRL_flipped] @ W_ho.T + b_ho;
log_softmax over vocab 32000. Output [128, 32, 32000] f32 (~524 MB).

Distribution: data-parallel over the 4096 flat (seq*batch) positions,
512 per core. The tiny recurrence is replicated on every core; each core
then computes logits + log_softmax for its position range only, selected
at runtime via partition_id() dynamic slices (no collectives needed --
softmax reduces over vocab, which is core-local).

log_softmax max-subtraction is dropped: logits are bounded (~|5|), so
f32 exp cannot overflow; out = logits - ln(sum exp(logits)).

Host-side work is limited to layout transforms (transposes, bias-row
augmentation) and the embedding row gather; all arithmetic (projections,
recurrences, logits matmul, softmax) runs on the NeuronCores.
"""

import os

import numpy as np
import ml_dtypes

SEQ, B, VOCAB = 128, 32, 32000
EMB, HID = 32, 16
NCORES = 8
POS = SEQ * B                 # 4096 flat positions, f = s*B + b
PPC = POS // NCORES           # 512 positions per core
PTILES = PPC // 128           # 4 position tiles of 128 per core
KDIM = 2 * HID + 1            # 33: [hLR; hRL; ones] contraction dim
XDIM = EMB + 1                # 33: [emb; ones] rows for x-projection
GW1 = 1536                    # pass-1 PSUM group width (3 banks)
G1 = [(g * GW1, GW1) for g in range(20)] + [(20 * GW1, VOCAB - 20 * GW1)]
SW = 4096                     # pass-2 SBUF staging stripe width
STRIPES = [(s * SW, min(SW, VOCAB - s * SW)) for s in range((VOCAB + SW - 1) // SW)]


def _mm_splits(w):
    out = []
    j = 0
    while j < w:
        jw = min(512, w - j)
        out.append((j, jw))
        j += jw
    return out


_CACHE = {}


def _build():
    if "nc" in _CACHE:
        return _CACHE["nc"]

    import concourse.bass as bass
    import concourse.tile as tile
    from concourse import bacc, mybir

    f32 = mybir.dt.float32
    bf16 = mybir.dt.bfloat16
    AF = mybir.ActivationFunctionType

    nc = bacc.Bacc(
        "TRN2",
        target_bir_lowering=False,
        debug=False,
        num_devices=NCORES,
    )

    d_embT = nc.dram_tensor("embT", [XDIM, POS], f32, kind="ExternalInput").ap()
    d_h0lrT = nc.dram_tensor("h0lrT", [HID, B], f32, kind="ExternalInput").ap()
    d_h0rlT = nc.dram_tensor("h0rlT", [HID, B], f32, kind="ExternalInput").ap()
    d_wxlr = nc.dram_tensor("wxlr", [XDIM, HID], f32, kind="ExternalInput").ap()
    d_whlr = nc.dram_tensor("whlr", [HID, HID], f32, kind="ExternalInput").ap()
    d_wxrl = nc.dram_tensor("wxrl", [XDIM, HID], f32, kind="ExternalInput").ap()
    d_whrl = nc.dram_tensor("whrl", [HID, HID], f32, kind="ExternalInput").ap()
    d_who = nc.dram_tensor("who", [KDIM, VOCAB], bf16, kind="ExternalInput").ap()
    d_out = nc.dram_tensor("out", [PPC, VOCAB], f32, kind="ExternalOutput").ap()

    with tile.TileContext(nc) as tc:
        with tc.tile_pool(name="const", bufs=1) as cpool:
            embT_s = cpool.tile([XDIM, POS], f32)
            wxlr_s = cpool.tile([XDIM, HID], f32)
            whlr_s = cpool.tile([HID, HID], f32)
            wxrl_s = cpool.tile([XDIM, HID], f32)
            whrl_s = cpool.tile([HID, HID], f32)
            who_s = cpool.tile([KDIM, VOCAB], bf16)
            # Both chains' states interleaved at 32-col granularity:
            # col block 2k   = hLR state k (position-indexed)
            # col block 2k+1 = hRL state k (STEP-indexed; position s uses
            #                  step 127-s)
            # so each step's two matmul pairs fill adjacent columns of ONE
            # [16, 64] PSUM tile and a single tanh covers both chains.
            comb = cpool.tile([HID, 2 * POS], f32)

            nc.sync.dma_start(embT_s[:], d_embT[:])
            nc.sync.dma_start(wxlr_s[:], d_wxlr[:])
            nc.sync.dma_start(whlr_s[:], d_whlr[:])
            nc.sync.dma_start(wxrl_s[:], d_wxrl[:])
            nc.sync.dma_start(whrl_s[:], d_whrl[:])
            nc.sync.dma_start(who_s[:], d_who[:])
            nc.sync.dma_start(comb[:, 0:B], d_h0lrT[:])
            nc.sync.dma_start(comb[:, B : 2 * B], d_h0rlT[:])

            # ---- Recurrences (replicated on every core) ----
            # Step k: LR consumes emb[k], RL consumes emb[127-k]; both write
            # state k+1. Note mm order: the LR group must close (stop=True)
            # before RL's start=True clears the bank's has_written bits.
            # The two chains use SEPARATE PSUM banks of one [16, 1024] tile
            # (cols 0:32 = LR in bank 0, cols 512:544 = RL in bank 1), so the
            # chain-independent x-projection matmuls of BOTH chains run ahead
            # of the tanh->h-matmul dependency chain; the merged tanh reads
            # both banks with one strided 3-D access pattern.
            with tc.tile_pool(name="recpsum", bufs=4, space="PSUM") as rpsum:
                for k in range(SEQ - 1):
                    r = SEQ - 1 - k
                    pk = rpsum.tile([HID, 1024], f32, tag="rp")
                    nc.tensor.matmul(
                        pk[:, 0:B],
                        lhsT=wxlr_s[:],
                        rhs=embT_s[:, k * B : (k + 1) * B],
                        start=True,
                        stop=False,
                    )
                    nc.tensor.matmul(
                        pk[:, 512 : 512 + B],
                        lhsT=wxrl_s[:],
                        rhs=embT_s[:, r * B : (r + 1) * B],
                        start=True,
                        stop=False,
                    )
                    nc.tensor.matmul(
                        pk[:, 0:B],
                        lhsT=whlr_s[:],
                        rhs=comb[:, 2 * k * B : (2 * k + 1) * B],
                        start=False,
                        stop=True,
                    )
                    nc.tensor.matmul(
                        pk[:, 512 : 512 + B],
                        lhsT=whrl_s[:],
                        rhs=comb[:, (2 * k + 1) * B : (2 * k + 2) * B],
                        start=False,
                        stop=True,
                    )
                    pk3 = pk[:].rearrange("p (g c) -> p g c", c=512)[:, :, 0:B]
                    out3 = comb[
                        :, (2 * k + 2) * B : (2 * k + 4) * B
                    ].rearrange("p (g c) -> p g c", c=B)
                    nc.scalar.activation(out3, pk3, AF.Tanh)

            # state view [16, 128 states, 64]; cols 0:32 = hLR, 32:64 = hRL
            comb3 = comb[:].rearrange("p (k c) -> p k c", c=2 * B)

            # ---- Output stage: this core's 512 positions ----
            pid = nc.partition_id()
            with (
                tc.tile_pool(name="bigpsum", bufs=2, space="PSUM") as bpsum,
                tc.tile_pool(name="stagep", bufs=3) as stpool,
                tc.tile_pool(name="outst", bufs=4) as opool,
                tc.tile_pool(name="smalls", bufs=2) as smpool,
            ):
                stages = [None] * PTILES
                negs = [None] * PTILES

                def build_stage(ppt):
                    # Compute engines can't target a partition base of 16, so
                    # the hRL rows go through a tmp tile + SBUF->SBUF DMA.
                    # Copies read comb (f32) directly, casting to bf16.
                    stage = stpool.tile([KDIM, 128], bf16, tag="stage")
                    tmpRL = stpool.tile([HID, 128], bf16, tag="tmpRL")
                    nc.vector.tensor_copy(
                        stage[0:HID, :].rearrange("p (k c) -> p k c", c=B),
                        comb3[:, bass.ts(pid * PTILES + ppt, 4), 0:B],
                    )
                    # position s = pid*16 + ppt*4 + i uses hRL step 127-s
                    for i in range(4):
                        s_hi = SEQ - 1 - ppt * 4 - i
                        nc.vector.tensor_copy(
                            tmpRL[
                                :, i * B : (i + 1) * B
                            ].rearrange("p (k c) -> p k c", c=B),
                            comb3[:, bass.ds(s_hi - pid * 16, 1), B : 2 * B],
                        )
                    nc.sync.dma_start(stage[HID : 2 * HID, :], tmpRL[:])
                    nc.vector.memset(stage[2 * HID : KDIM, :], 1.0)
                    stages[ppt] = stage

                def pass1(ppt):
                    stage = stages[ppt]
                    sums = smpool.tile([128, len(G1)], f32, tag="sums")
                    for g, (c0, w) in enumerate(G1):
                        pt = bpsum.tile([128, GW1], f32, tag="p1")
                        for j0, jw in _mm_splits(w):
                            nc.tensor.matmul(
                                pt[:, j0 : j0 + jw],
                                lhsT=stage[:],
                                rhs=who_s[:, c0 + j0 : c0 + j0 + jw],
                                start=True,
                                stop=True,
                            )
                        nc.scalar.activation(
                            pt[:, :w],
                            pt[:, :w],
                            AF.Exp,
                            accum_out=sums[:, g : g + 1],
                        )
                    return sums

                def reduce_ln(ppt, sums):
                    # -ln(S) entirely on DVE so ACT never leaves the
                    # exp/tanh table set (each ACT Ln forced a ~1.3us
                    # table-set reload between exp batches).
                    # S = m * 2^e, m in [1,2):
                    #   -ln(S) = -e*ln2 - ln(m), ln(m) via minimax poly.
                    S = smpool.tile([128, 1], f32, tag="S")
                    nc.vector.tensor_reduce(
                        S[:],
                        sums[:],
                        axis=mybir.AxisListType.X,
                        op=mybir.AluOpType.add,
                    )
                    i32 = mybir.dt.int32
                    bits = smpool.tile([128, 1], i32, tag="bits")
                    nc.vector.tensor_scalar(
                        bits[:],
                        S[:].bitcast(i32),
                        23,
                        None,
                        mybir.AluOpType.logical_shift_right,
                    )
                    nc.vector.tensor_scalar_add(bits[:], bits[:], -127)
                    e_f = smpool.tile([128, 1], f32, tag="e_f")
                    nc.vector.tensor_copy(e_f[:], bits[:])  # int -> float
                    mant = smpool.tile([128, 1], i32, tag="mant")
                    nc.vector.tensor_scalar(
                        mant[:],
                        S[:].bitcast(i32),
                        0x007FFFFF,
                        0x3F800000,
                        mybir.AluOpType.bitwise_and,
                        mybir.AluOpType.bitwise_or,
                    )
                    m = mant[:].bitcast(f32)
                    # ln(m) on [1,2): degree-5 poly in t = m - 1 for
                    # ln(1+t)/t, lstsq fit, |err on ln(m)| < 4e-6.
                    t = smpool.tile([128, 1], f32, tag="t")
                    nc.vector.tensor_scalar_add(t[:], m, -1.0)
                    C = [0.99987663, -0.49760941, 0.31669577,
                         -0.19225670, 0.08450634, -0.01806849]
                    acc = smpool.tile([128, 1], f32, tag="acc")
                    nc.vector.tensor_scalar(
                        acc[:], t[:], C[5], C[4],
                        mybir.AluOpType.mult, mybir.AluOpType.add,
                    )
                    for c in (C[3], C[2], C[1], C[0]):
                        nc.vector.tensor_tensor(
                            acc[:], acc[:], t[:], mybir.AluOpType.mult
                        )
                        nc.vector.tensor_scalar_add(acc[:], acc[:], c)
                    # acc ~= ln(m)/t ; neg = -(e*ln2 + t*acc)
                    nc.vector.tensor_tensor(acc[:], acc[:], t[:], mybir.AluOpType.mult)
                    neg = smpool.tile([128, 1], f32, tag="neg")
                    nc.vector.tensor_scalar(
                        neg[:], e_f[:], float(np.log(2.0)), None,
                        mybir.AluOpType.mult,
                    )
                    nc.vector.tensor_tensor(neg[:], neg[:], acc[:], mybir.AluOpType.add)
                    nc.vector.tensor_scalar_mul(neg[:], neg[:], -1.0)
                    negs[ppt] = neg

                def pass2(ppt):
                    stage = stages[ppt]
                    neg = negs[ppt]
                    gi = 0
                    for s0, sw in STRIPES:
                        ot = opool.tile([128, SW], f32, tag="ot")
                        for j0, jw in _mm_splits(sw):
                            pt2 = bpsum.tile([128, 512], f32, tag="p2")
                            nc.tensor.matmul(
                                pt2[:, :jw],
                                lhsT=stage[:],
                                rhs=who_s[:, s0 + j0 : s0 + j0 + jw],
                                start=True,
                                stop=True,
                            )
                            nc.vector.tensor_scalar_add(
                                ot[:, j0 : j0 + jw], pt2[:, :jw], neg[:, 0:1]
                            )
                            gi += 1
                        nc.sync.dma_start(
                            d_out[ppt * 128 : (ppt + 1) * 128, s0 : s0 + sw],
                            ot[:, :sw],
                        )

                for ppt in range(PTILES):
                    build_stage(ppt)
                    sums = pass1(ppt)
                    if ppt > 0:
                        pass2(ppt - 1)
                    reduce_ln(ppt, sums)
                pass2(PTILES - 1)

    nc.compile()
    _CACHE["nc"] = nc
    return nc


def _prep(inputs):
    f32 = np.float32
    ids = np.asarray(inputs["input_batch"]).reshape(-1).astype(np.int64)
    emb = np.asarray(inputs["embedding"], dtype=f32)[ids]  # [4096, 32]

    embT = np.empty((XDIM, POS), f32)
    embT[:EMB] = emb.T
    embT[EMB] = 1.0

    def aug_x(W, b):
        out = np.empty((XDIM, HID), f32)
        out[:EMB] = np.asarray(W, dtype=f32)[:, :EMB].T
        out[EMB] = np.asarray(b, dtype=f32)
        return out

    W_lr = np.asarray(inputs["W_lr"], dtype=f32)
    W_rl = np.asarray(inputs["W_rl"], dtype=f32)
    who = np.empty((KDIM, VOCAB), f32)
    who[: 2 * HID] = np.asarray(inputs["W_ho"], dtype=f32).T
    who[2 * HID] = np.asarray(inputs["b_ho"], dtype=f32)

    return {
        "embT": embT,
        "h0lrT": np.ascontiguousarray(np.asarray(inputs["h0_lr"], dtype=f32).T),
        "h0rlT": np.ascontiguousarray(np.asarray(inputs["h0_rl"], dtype=f32).T),
        "wxlr": aug_x(W_lr, inputs["b_lr"]),
        "whlr": np.ascontiguousarray(W_lr[:, EMB:].T),
        "wxrl": aug_x(W_rl, inputs["b_rl"]),
        "whrl": np.ascontiguousarray(W_rl[:, EMB:].T),
        "who": who.astype(ml_dtypes.bfloat16),
    }


LAST_RESULTS = None


def kernel(**inputs):
    from concourse.bass_utils import run_bass_kernel_spmd

    nc = _build()
    in_map = _prep(inputs)
    trace = bool(int(os.environ.get("BASS_KERNEL_TRACE", "0")))
    res = run_bass_kernel_spmd(
        nc,
        [in_map] * NCORES,
        list(range(NCORES)),
        trace=trace,
    )
    global LAST_RESULTS
    LAST_RESULTS = res
    out = np.concatenate([res.results[c]["out"] for c in range(NCORES)], axis=0)
    return np.ascontiguousarray(out.reshape(SEQ, B, VOCAB).astype(np.float32))

